# revision 9
# baseline (speedup 1.0000x reference)
"""PointNetFeaturePropagation kernel for 8 trn2 NeuronCores.

Sharding: core = (b, h); b = core // 2 batch, h = core % 2 half of N.
Each core: 4096 query points, full S=2048 source points of its batch.
"""

import numpy as np

import concourse.mybir as mybir
import concourse.tile as tile
from concourse import bacc, bass
from concourse.bass_utils import run_bass_kernel_spmd

B, N, S = 4, 8192, 2048
D1, D2 = 256, 512
NSH = N // 2  # rows per core
NT = NSH // 128  # 32 n-tiles
NCHUNK = 512  # MLP free-dim chunk
NCH = NSH // NCHUNK  # 8 chunks
BN_EPS = 1e-5
NTOT = float(B * N)  # BN stat divisor

F32 = mybir.dt.float32
BF16 = mybir.dt.bfloat16
U16 = mybir.dt.uint16
I16 = mybir.dt.int16
BFNP = mybir.dt.np(BF16)

_NC_CACHE = {}


def build_nc(repeat=1, debug=False):
    key = (repeat, debug)
    if key in _NC_CACHE:
        return _NC_CACHE[key]
    nc = bacc.Bacc(trn_type="TRN2", num_devices=8)

    # per-core inputs
    dlhs = nc.dram_tensor("dlhs", [21, NSH], BF16, kind="ExternalInput")
    drhs = nc.dram_tensor("drhs", [21, S], BF16, kind="ExternalInput")
    sq1e = nc.dram_tensor("sq1e", [128, NT], F32, kind="ExternalInput")
    p2tab = nc.dram_tensor("p2tab", [S, D2], BF16, kind="ExternalInput")
    p1s = nc.dram_tensor("p1s", [128, 2, NSH], BF16, kind="ExternalInput")
    w0t = nc.dram_tensor("w0t", [128, 6, 256], BF16, kind="ExternalInput")
    w1t = nc.dram_tensor("w1t", [128, 2, 256], BF16, kind="ExternalInput")
    bn0 = nc.dram_tensor("bn0", [128, 2, 2], F32, kind="ExternalInput")
    bn1 = nc.dram_tensor("bn1", [128, 2, 2], F32, kind="ExternalInput")

    yout = nc.dram_tensor("yout", [128, 2, NSH], F32, kind="ExternalOutput")
    if debug:
        dbg_mv = nc.dram_tensor("dbg_mv", [128, NT, 8], F32, kind="ExternalOutput")
        dbg_mi = nc.dram_tensor("dbg_mi", [128, NT, 8], U16, kind="ExternalOutput")
        dbg_wn = nc.dram_tensor("dbg_wn", [128, NT, 3], F32, kind="ExternalOutput")
        dbg_idxw = nc.dram_tensor("dbg_idxw", [128, 3, NSH // 16], I16, kind="ExternalOutput")
        dbg_acc = nc.dram_tensor("dbg_acc", [128, 4, D2], BF16, kind="ExternalOutput")
        dbg_feat = nc.dram_tensor("dbg_feat", [128, 6, NCHUNK], BF16, kind="ExternalOutput")
        dbg_y0 = nc.dram_tensor("dbg_y0", [128, 2, NSH], BF16, kind="ExternalOutput")
        dbg_st0 = nc.dram_tensor("dbg_st0", [128, 2, 2, NCH], F32, kind="ExternalOutput")
        dbg_sc0 = nc.dram_tensor("dbg_sc0", [128, 2], F32, kind="ExternalOutput")
        dbg_tb0 = nc.dram_tensor("dbg_tb0", [128, 2], F32, kind="ExternalOutput")

    idxtmp = nc.dram_tensor("idxtmp", [3, NSH], U16, kind="Internal")
    idxrep = nc.dram_tensor("idxrep", [3, 16, NSH // 16], U16, kind="Internal")

    with tile.TileContext(nc) as tc:
        with (
            tc.tile_pool(name="persist", bufs=1) as pp,
            tc.tile_pool(name="feat", bufs=1) as fp,
            tc.tile_pool(name="dram", bufs=2, space="DRAM") as dp,
            tc.tile_pool(name="small", bufs=2) as sp,
        ):
            for _rep in range(repeat):
                dlhs_t = pp.tile([21, NSH], BF16, tag="dlhs")
                nc.sync.dma_start(out=dlhs_t, in_=dlhs[:, :])
                drhs_t = pp.tile([21, S], BF16, tag="drhs")
                nc.sync.dma_start(out=drhs_t, in_=drhs[:, :])
                sq1e_t = pp.tile([128, NT], F32, tag="sq1e")
                nc.sync.dma_start(out=sq1e_t, in_=sq1e[:, :])
                w0t_t = pp.tile([128, 6, 256], BF16, tag="w0t")
                nc.sync.dma_start(out=w0t_t, in_=w0t[:, :, :])
                w1t_t = pp.tile([128, 2, 256], BF16, tag="w1t")
                nc.sync.dma_start(out=w1t_t, in_=w1t[:, :, :])
                bn0_t = pp.tile([128, 2, 2], F32, tag="bn0")
                nc.sync.dma_start(out=bn0_t, in_=bn0[:, :, :])
                bn1_t = pp.tile([128, 2, 2], F32, tag="bn1")
                nc.sync.dma_start(out=bn1_t, in_=bn1[:, :, :])

                mv = pp.tile([128, NT, 8], F32, tag="mv")
                mi = pp.tile([128, NT, 8], U16, tag="mi")

                # --- phase 1: distance matmuls + top-8 scan ---
                with tc.tile_pool(name="ndps", bufs=2, space="PSUM") as ndp_pool:
                    for i in range(NT):
                        ndp = ndp_pool.tile([128, S], F32, tag="nd")
                        for j in range(4):
                            nc.tensor.matmul(
                                ndp[:, j * 512 : (j + 1) * 512],
                                dlhs_t[:, i * 128 : (i + 1) * 128],
                                drhs_t[:, j * 512 : (j + 1) * 512],
                                start=True,
                                stop=True,
                            )
                        nc.vector.max(out=mv[:, i, :], in_=ndp)
                        nc.vector.max_index(mi[:, i, :], mv[:, i, :], ndp)

                # --- phase 1.5: weights (batched small ops) ---
                # d_k + 1e-8 = sq1e - m_k ; r = 1/d ; wn = r / (r0+r1+r2)
                dv = sp.tile([128, NT, 3], F32, tag="dv")
                sq_b = bass.AP(
                    tensor=sq1e_t.tensor,
                    offset=sq1e_t.offset,
                    ap=[sq1e_t.ap[0], sq1e_t.ap[1], [0, 3]],
                )
                nc.vector.tensor_sub(dv, sq_b, mv[:, :, 0:3])
                # guard: exact-duplicate points cancel to 0 -> clamp like
                # the reference's d+1e-8 (weight ~= 1 for that neighbor)
                nc.vector.tensor_scalar_max(dv, dv, 1e-8)
                rec = sp.tile([128, NT, 3], F32, tag="rec")
                nc.vector.reciprocal(rec, dv)
                zt = sp.tile([128, NT], F32, tag="zt")
                nc.vector.tensor_add(zt, rec[:, :, 0], rec[:, :, 1])
                nc.vector.tensor_add(zt, zt, rec[:, :, 2])
                rz = sp.tile([128, NT], F32, tag="rz")
                nc.vector.reciprocal(rz, zt)
                wn = pp.tile([128, NT, 3], F32, tag="wn")
                rz_b = bass.AP(
                    tensor=rz.tensor,
                    offset=rz.offset,
                    ap=[rz.ap[0], rz.ap[1], [0, 3]],
                )
                nc.vector.tensor_mul(wn, rec, rz_b)

                # --- phase 2: stage indices to wrapped-16 layout ---
                idxw = pp.tile([128, 3, NSH // 16], I16, tag="idxw")
                for k in range(3):
                    nc.sync.dma_start(
                        out=idxtmp[k : k + 1, :].rearrange(
                            "one (c p) -> (one p) c", p=128
                        ),
                        in_=mi[:, :, k],
                    )
                    # flat [NSH] -> wrap16 [16, NSH//16] (a 16xF transpose;
                    # tiny data, 1-elem descriptors are fine)
                    src = bass.AP(
                        tensor=idxtmp,
                        offset=k * NSH,
                        ap=[[16, NSH // 16], [1, 16]],
                    )
                    dst = bass.AP(
                        tensor=idxrep,
                        offset=k * NSH,
                        ap=[[1, NSH // 16], [NSH // 16, 16]],
                    )
                    with nc.allow_non_contiguous_dma(
                        reason="16-wrap transpose, 8KB total"
                    ):
                        nc.sync.dma_start(out=dst, in_=src)
                    for j in range(8):
                        nc.sync.dma_start(
                            out=idxw[16 * j : 16 * (j + 1), k, :],
                            in_=idxrep[k, :, :].bitcast(I16),
                        )

                if debug:
                    nc.sync.dma_start(out=dbg_mv[:, :, :], in_=mv)
                    nc.sync.dma_start(out=dbg_mi[:, :, :], in_=mi)
                    nc.sync.dma_start(out=dbg_wn[:, :, :], in_=wn)
                    nc.sync.dma_start(out=dbg_idxw[:, :, :], in_=idxw)

                # --- phases 3-6 chunked: gather, wsum, transpose, MLP0 ---
                CPT = NCHUNK // 128  # tiles per chunk (4)
                y0sb = fp.tile([128, 2, NSH], BF16, tag="y0sb")
                st0 = sp.tile([128, 2, 2, NCH], F32, tag="st0")
                dump = sp.tile([128, NCHUNK], BF16, tag="dump")
                with tc.tile_pool(name="gpool", bufs=2) as gp, tc.tile_pool(
                    name="mlpps", bufs=4, space="PSUM"
                ) as mp:
                    for ch in range(NCH):
                        cs = slice(ch * NCHUNK, (ch + 1) * NCHUNK)
                        gt = []
                        for k in range(3):
                            g = gp.tile([128, CPT, D2], BF16, tag=f"g{k}")
                            nc.gpsimd.dma_gather(
                                g[:],
                                p2tab[:, :],
                                idxw[:, k, ch * (NCHUNK // 16):(ch + 1) * (NCHUNK // 16)],
                                NCHUNK,
                                NCHUNK,
                                D2,
                            )
                            gt.append(g)
                        for c in range(CPT):
                            ci = ch * CPT + c
                            for k in range(3):
                                nc.scalar.activation(
                                    gt[k][:, c, :],
                                    gt[k][:, c, :],
                                    mybir.ActivationFunctionType.Copy,
                                    scale=wn[:, ci, k : k + 1],
                                )
                        nc.vector.tensor_add(gt[0][:], gt[0][:], gt[1][:])
                        nc.vector.tensor_add(gt[0][:], gt[0][:], gt[2][:])

                        if debug and ch == 0:
                            nc.sync.dma_start(out=dbg_acc[:, :, :], in_=gt[0][:])
                        feat = gp.tile([128, 6, NCHUNK], BF16, tag="feat")
                        nc.sync.dma_start(
                            out=feat[:, 0:2, :], in_=p1s[:, :, cs]
                        )
                        for c in range(CPT):
                            nc.sync.dma_start_transpose(
                                feat[:, 2:6, c * 128 : (c + 1) * 128],
                                gt[0][:, c, :],
                            )

                        if debug and ch == 0:
                            nc.sync.dma_start(out=dbg_feat[:, :, :], in_=feat[:])
                        for m in range(2):
                            yp = mp.tile([128, NCHUNK], F32, tag="yp")
                            for k in range(6):
                                nc.tensor.matmul(
                                    yp,
                                    w0t_t[:, k, m * 128 : (m + 1) * 128],
                                    feat[:, k, :],
                                    start=(k == 0),
                                    stop=(k == 5),
                                )
                            nc.scalar.activation(
                                y0sb[:, m, cs],
                                yp,
                                mybir.ActivationFunctionType.Copy,
                                accum_out=st0[:, m, 0, ch : ch + 1],
                            )
                            nc.scalar.activation(
                                dump,
                                yp,
                                mybir.ActivationFunctionType.Square,
                                accum_out=st0[:, m, 1, ch : ch + 1],
                            )

                    if debug:
                        nc.sync.dma_start(out=dbg_y0[:, :, :], in_=y0sb)
                        nc.sync.dma_start(out=dbg_st0[:, :, :, :], in_=st0)

                    # --- phase 7: allreduce stats 0, bn params ---
                    sc0, tb0 = _bn_reduce(nc, tc, sp, dp, st0, bn0_t, "0")
                    if debug:
                        nc.sync.dma_start(out=dbg_sc0[:, :], in_=sc0)
                        nc.sync.dma_start(out=dbg_tb0[:, :], in_=tb0)

                    # --- phase 8: bn apply + relu ---
                    y0n = fp.tile([128, 2, NSH], BF16, tag="y0n")
                    for m in range(2):
                        nc.scalar.activation(
                            y0n[:, m, :],
                            y0sb[:, m, :],
                            mybir.ActivationFunctionType.Relu,
                            bias=tb0[:, m : m + 1],
                            scale=sc0[:, m : m + 1],
                        )

                    # --- phase 9: MLP layer 1 + stats ---
                    st1 = sp.tile([128, 2, 2, NCH], F32, tag="st1")
                    y1sb = fp.tile([128, 2, NSH], BF16, tag="y1sb")
                    for ch in range(NCH):
                        cs = slice(ch * NCHUNK, (ch + 1) * NCHUNK)
                        for m in range(2):
                            yp = mp.tile([128, NCHUNK], F32, tag="yp")
                            for k in range(2):
                                nc.tensor.matmul(
                                    yp,
                                    w1t_t[:, k, m * 128 : (m + 1) * 128],
                                    y0n[:, k, cs],
                                    start=(k == 0),
                                    stop=(k == 1),
                                )
                            nc.scalar.activation(
                                y1sb[:, m, cs],
                                yp,
                                mybir.ActivationFunctionType.Copy,
                                accum_out=st1[:, m, 0, ch : ch + 1],
                            )
                            nc.scalar.activation(
                                dump,
                                yp,
                                mybir.ActivationFunctionType.Square,
                                accum_out=st1[:, m, 1, ch : ch + 1],
                            )

                    sc1, tb1 = _bn_reduce(nc, tc, sp, dp, st1, bn1_t, "1")

                    # --- phase 10: final bn apply + relu + out ---
                    for m in range(2):
                        yo = sp.tile([128, NSH], F32, tag="yo")
                        nc.scalar.activation(
                            yo,
                            y1sb[:, m, :],
                            mybir.ActivationFunctionType.Relu,
                            bias=tb1[:, m : m + 1],
                            scale=sc1[:, m : m + 1],
                        )
                        nc.sync.dma_start(out=yout[:, m, :], in_=yo)

    nc.compile()
    _NC_CACHE[key] = nc
    return nc


def _bn_reduce(nc, tc, sp, dp, st, bn_t, suffix):
    """st [128, 2, 2, NCH] chunk partials -> allreduce -> scale/bias."""
    ar = sp.tile([128, 4], F32, tag=f"ar{suffix}")
    nc.vector.reduce_sum(
        ar.rearrange("p (a b) -> p a b", b=1),
        st.rearrange("p a b c -> p (a b) c"),
        axis=mybir.AxisListType.X,
    )
    bi = dp.tile([128, 4], F32, tag=f"bi{suffix}")
    bo = dp.tile([128, 4], F32, tag=f"bo{suffix}")
    nc.gpsimd.dma_start(bi[:], ar[:])
    nc.gpsimd.collective_compute(
        "AllReduce",
        mybir.AluOpType.add,
        replica_groups=[list(range(8))],
        ins=[bi.opt()],
        outs=[bo.opt()],
    )
    arg = sp.tile([128, 2, 2], F32, tag=f"arg{suffix}")
    nc.gpsimd.dma_start(arg.rearrange("p a b -> p (a b)"), bo[:])
    # mean = sum/NTOT ; msq = sq/NTOT ; var = msq - mean^2
    mean = sp.tile([128, 2], F32, tag=f"mean{suffix}")
    nc.vector.tensor_scalar_mul(mean, arg[:, :, 0], 1.0 / NTOT)
    msq = sp.tile([128, 2], F32, tag=f"msq{suffix}")
    nc.vector.tensor_scalar_mul(msq, arg[:, :, 1], 1.0 / NTOT)
    var = sp.tile([128, 2], F32, tag=f"var{suffix}")
    nc.vector.tensor_mul(var, mean, mean)
    nc.vector.tensor_sub(var, msq, var)
    nc.vector.tensor_scalar_add(var, var, float(BN_EPS))
    sd = sp.tile([128, 2], F32, tag=f"sd{suffix}")
    nc.scalar.activation(sd, var, mybir.ActivationFunctionType.Sqrt)
    rsd = sp.tile([128, 2], F32, tag=f"rsd{suffix}")
    nc.vector.reciprocal(rsd, sd)
    # s = gamma * rsd ; t = beta - mean * s
    sc = sp.tile([128, 2], F32, tag=f"sc{suffix}")
    nc.vector.tensor_mul(sc, bn_t[:, :, 0], rsd)
    tb = sp.tile([128, 2], F32, tag=f"tb{suffix}")
    nc.vector.tensor_mul(tb, mean, sc)
    nc.vector.tensor_sub(tb, bn_t[:, :, 1], tb)
    return sc, tb


def _split3(x):
    a = x.astype(BFNP).astype(np.float32)
    r = x - a
    b = r.astype(BFNP).astype(np.float32)
    c = (r - b).astype(BFNP)
    return a.astype(BFNP), b.astype(BFNP), c


def _prep_core(xyz1h, xyz2b, p1h, p2b, W0, W1, g0, b0, g1, b1):
    """Host prep for one core. xyz1h [NSH,3], xyz2b [S,3], p1h [D1,NSH],
    p2b [D2,S]."""
    x2p = (2.0 * xyz2b).astype(np.float32)
    a1, b1_, c1 = _split3(xyz1h.astype(np.float32))
    a2, b2_, c2 = _split3(x2p)
    pairs = [(a1, a2), (a1, b2_), (b1_, a2), (a1, c2), (c1, a2), (b1_, b2_)]
    dlhs = np.zeros((21, NSH), BFNP)
    drhs = np.zeros((21, S), BFNP)
    for t, (u, v) in enumerate(pairs):
        dlhs[3 * t : 3 * t + 3, :] = u.T
        drhs[3 * t : 3 * t + 3, :] = v.T
    s2 = np.sum(xyz2b.astype(np.float32) ** 2, axis=1, dtype=np.float32)
    nh, nm, nl = _split3(-s2.astype(np.float32))
    drhs[18, :] = nh
    drhs[19, :] = nm
    drhs[20, :] = nl
    dlhs[18:21, :] = np.ones((3, NSH), BFNP)

    sq1 = np.sum(xyz1h.astype(np.float32) ** 2, axis=1, dtype=np.float32)
    sq1e = (sq1 + np.float32(1e-8)).reshape(NT, 128).T.copy()

    p2tab = np.ascontiguousarray(p2b.T).astype(BFNP)
    p1t = p1h.astype(BFNP).reshape(2, 128, NSH).transpose(1, 0, 2).copy()
    w0t = (
        W0.T.astype(BFNP).reshape(6, 128, 256).transpose(1, 0, 2).copy()
    )
    w1t = (
        W1.T.astype(BFNP).reshape(2, 128, 256).transpose(1, 0, 2).copy()
    )
    bn0 = np.stack([g0, b0], axis=1).reshape(2, 128, 2).transpose(1, 0, 2).copy()
    bn1 = np.stack([g1, b1], axis=1).reshape(2, 128, 2).transpose(1, 0, 2).copy()
    return {
        "dlhs": dlhs,
        "drhs": drhs,
        "sq1e": np.ascontiguousarray(sq1e),
        "p2tab": p2tab,
        "p1s": p1t,
        "w0t": w0t,
        "w1t": w1t,
        "bn0": np.ascontiguousarray(bn0.astype(np.float32)),
        "bn1": np.ascontiguousarray(bn1.astype(np.float32)),
    }


def kernel(xyz1, xyz2, points1, points2, W0, g0, b0, W1, g1, b1, _repeat=1):
    xyz1 = np.asarray(xyz1, np.float32)
    xyz2 = np.asarray(xyz2, np.float32)
    points1 = np.asarray(points1, np.float32)
    points2 = np.asarray(points2, np.float32)
    W0 = np.asarray(W0, np.float32)
    W1 = np.asarray(W1, np.float32)
    g0 = np.asarray(g0, np.float32)
    b0 = np.asarray(b0, np.float32)
    g1 = np.asarray(g1, np.float32)
    b1 = np.asarray(b1, np.float32)

    nc = build_nc(_repeat)
    in_maps = []
    for core in range(8):
        b, h = core // 2, core % 2
        sl = slice(h * NSH, (h + 1) * NSH)
        in_maps.append(
            _prep_core(
                xyz1[b, sl], xyz2[b], points1[b, :, sl], points2[b],
                W0, W1, g0, b0, g1, b1,
            )
        )
    res = run_bass_kernel_spmd(nc, in_maps, core_ids=list(range(8)))
    out = np.zeros((B, 256, N), np.float32)
    for core in range(8):
        b, h = core // 2, core % 2
        y = res.results[core]["yout"]  # [128, 2, NSH]
        out[b, :, h * NSH : (h + 1) * NSH] = y.transpose(1, 0, 2).reshape(
            256, NSH
        )
    return out
